# revision 5
# baseline (speedup 1.0000x reference)
"""Per-batch (block-diagonal) cross-attention kernel for Trainium2.

Each query row attends only to key/value rows with the same batch id
(ids in [0, 8), both coor arrays sorted). Batch b -> core b: every core
runs one dense attention block of ~1k queries x ~1k keys, C=64, fully
independent (no collectives).

Host-side sharding passes, per core (padded sizes Qp/Kp, multiples of 128):
  - qT [64, Qp], kT [64, Kp] : host-transposed Q/K, zero-padded, bf16
  - kv [128, nk*65]          : KV rows interleaved per k-tile; columns
                               [kti*65, kti*65+65) hold kv rows
                               {kti*128+p} with col 64 = 1.0 on valid
                               rows, 0 on padding, bf16

Device algorithm per core. HW constraint discovered on-device: a matmul
with start=True zeroes its ENTIRE 2KB PSUM bank (not just the output
region; CoreSim models region-only), so every concurrently-open
accumulation group must own a full bank. Budget of 8 banks:
  - S-main psum [P, QM=1024] x2 bufs        -> 4 banks
  - 4 live PV accumulators (j = 0..3)       -> 4 banks

Pipeline:
  1. Main spine, k-tile-outer: S-main(kti) on PE -> exp on ACT (the
     serial bottleneck; Scalar queue carries nothing else) -> pt tiles
     (bf16, one per k-tile, all kept in SBUF). PV for j0..3 accumulates
     per k-tile in its own bank. Emission order S(t+1) before PV(t)
     keeps PE fed while ACT works.
  2. q-tail pass (cols QM..Qp = exactly block j8): S-tail + exp-tail per
     k-tile, psums reusing the S-main pool slots (same tag). Runs on
     PE/ACT while...
  3. PV-rest: groups j4..7 (reusing the 4 PV banks, WAR-serialized on
     the finalize reads), then j8 after the tail exps.
  4. Finalize per j as its group closes: DVE reciprocal of the
     denominator (col 64, accumulated via the kv ones column) + ts_mul
     into the output tile; flushed in thirds on the SP ring.

Out layout [128, nq*64]: out row j*128+p lives at [p, j*64:(j+1)*64];
the host unpermutes. exp uses no max subtraction: randn scores are
O(1), exp cannot overflow, softmax is shift-invariant.
"""

import os
from contextlib import ExitStack

import numpy as np

import concourse.bacc as bacc
import concourse.bass as bass
import concourse.mybir as mybir
import concourse.tile as tile
from concourse.bass_utils import run_bass_kernel_spmd

N_CORES = 8
C = 64
P = 128
SCALE = 1.0 / 8.0  # 1/sqrt(C)
F32 = mybir.dt.float32

# Matmul dtype for the QK^T ("S") and PV stages: "f32", "f32r", "bf16".
S_MM = os.environ.get("XATTN_S_MM", "bf16")
PV_MM = os.environ.get("XATTN_PV_MM", "bf16")

_LAST_RUN = {}


def _round_up(x: int, m: int) -> int:
    return -(-x // m) * m


def _mm_cast(ap, mode: str):
    if mode == "f32r":
        return ap.bitcast(mybir.dt.float32r)
    return ap


def _emit(ctx: ExitStack, tc: "tile.TileContext", out_ap, qt_ap, kt_ap, kv_ap,
          Qp: int, Kp: int):
    nc = tc.nc
    nq, nk = Qp // P, Kp // P
    s_dt = mybir.dt.bfloat16 if S_MM == "bf16" else F32
    pv_dt = mybir.dt.bfloat16 if PV_MM == "bf16" else F32
    KW = C + 1  # kv tile width (values + ones column)
    QM = min(Qp, 1024)  # main S psum width (exactly 2 banks at 1024)
    QT = Qp - QM        # q tail = blocks j >= QM//P
    NL = min(4, nq)     # live PV groups during the main spine
    Exp = mybir.ActivationFunctionType.Exp

    big = ctx.enter_context(tc.tile_pool(name="big", bufs=1))
    psm = ctx.enter_context(tc.tile_pool(name="psm", bufs=2, space="PSUM"))
    pso = ctx.enter_context(tc.tile_pool(name="pso", bufs=NL, space="PSUM"))
    ptp = ctx.enter_context(tc.tile_pool(name="ptp", bufs=nk))
    outp = ctx.enter_context(tc.tile_pool(name="outp", bufs=2))

    qt = big.tile([C, Qp], s_dt, tag="qt", name="qt")
    kt = big.tile([C, Kp], s_dt, tag="kt", name="kt")
    kv = big.tile([P, nk * KW], pv_dt, tag="kv", name="kv")

    # DMA rings (only SP/ACT/gpsimd can initiate DMAs): SP carries the
    # head k-tile, the kT bulk, and the q tail; ACT carries one early
    # dispatch for the main q block (before the exp table load); kv
    # rides gpsimd/SWDGE. Everything lands before its first consumer.
    nc.sync.dma_start(kt[:, 0:P], kt_ap[:, 0:P])
    nc.scalar.dma_start(qt[:, 0:QM], qt_ap[:, 0:QM])
    nc.sync.dma_start(kt[:, P:Kp], kt_ap[:, P:Kp])
    if QT:
        nc.sync.dma_start(qt[:, QM:Qp], qt_ap[:, QM:Qp])
    nc.gpsimd.dma_start(kv[:], kv_ap[:, :])

    ot = big.tile([P, nq * C], F32, tag="ot", name="ot")

    pt_tiles = [None] * nk
    po_tiles = [None] * nq

    def emit_s_main(kti: int):
        ktile = _mm_cast(kt[:, kti * P:(kti + 1) * P], S_MM)
        ps = psm.tile([P, QM], F32, tag="ps", name="ps")
        for ch in range(0, QM, 512):
            w = min(512, QM - ch)
            nc.tensor.matmul(ps[:, ch:ch + w], lhsT=ktile,
                             rhs=_mm_cast(qt[:, ch:ch + w], S_MM),
                             start=True, stop=True)
        pt = ptp.tile([P, Qp], pv_dt, tag="pt", name="pt")
        pt_tiles[kti] = pt
        nc.scalar.activation(pt[:, 0:QM], ps[:], Exp, scale=SCALE)

    def emit_s_tail(kti: int):
        ktile = _mm_cast(kt[:, kti * P:(kti + 1) * P], S_MM)
        ps = psm.tile([P, QT], F32, tag="ps", name="pst")
        nc.tensor.matmul(ps[:], lhsT=ktile,
                         rhs=_mm_cast(qt[:, QM:Qp], S_MM),
                         start=True, stop=True)
        nc.scalar.activation(pt_tiles[kti][:, QM:Qp], ps[:], Exp, scale=SCALE)

    def emit_pv(j: int, kti: int):
        if kti == 0:
            po_tiles[j] = pso.tile([P, KW], F32, tag="po", name="po")
        nc.tensor.matmul(
            po_tiles[j][:],
            lhsT=_mm_cast(pt_tiles[kti][:, j * P:(j + 1) * P], PV_MM),
            rhs=_mm_cast(kv[:, kti * KW:(kti + 1) * KW], PV_MM),
            start=(kti == 0),
            stop=(kti == nk - 1),
        )

    def finalize(j: int):
        po = po_tiles[j]
        rec = outp.tile([P, 1], F32, tag="rec", name="rec")
        nc.vector.reciprocal(rec[:], po[:, C:C + 1])
        nc.vector.tensor_scalar_mul(ot[:, j * C:(j + 1) * C], po[:, 0:C], rec[:])

    # Main spine: S(t+1) emitted before PV-live(t).
    emit_s_main(0)
    for kti in range(1, nk):
        emit_s_main(kti)
        for j in range(NL):
            emit_pv(j, kti - 1)
    for j in range(NL):
        emit_pv(j, nk - 1)

    # q-tail pass (PE/ACT) — overlaps PV-rest below on the PE queue.
    if QT:
        for kti in range(nk):
            emit_s_tail(kti)

    for j in range(NL):
        finalize(j)

    flush = sorted({nq // 3 - 1, 2 * (nq // 3) - 1, nq - 1} if nq >= 3 else {nq - 1})
    prev = 0
    for j in range(nq):
        if j >= NL:
            for kti in range(nk):
                emit_pv(j, kti)
            finalize(j)
        if j in flush:
            nc.sync.dma_start(out_ap[:, prev * C:(j + 1) * C],
                              ot[:, prev * C:(j + 1) * C])
            prev = j + 1


def build_program(Qp: int, Kp: int):
    # Bacc (not bare Bass): its compile() legalizes sync waits for walrus
    # (at most one wait per instruction on TRN2).
    nc = bacc.Bacc(
        trn_type="TRN2",
        target_bir_lowering=False,
        debug=False,
        num_devices=N_CORES,
    )
    nk = Kp // P
    nq = Qp // P
    io_dt = mybir.dt.bfloat16 if S_MM == "bf16" else F32
    pv_dt = mybir.dt.bfloat16 if PV_MM == "bf16" else F32
    qt_ap = nc.dram_tensor("qT", [C, Qp], io_dt, kind="ExternalInput").ap()
    kt_ap = nc.dram_tensor("kT", [C, Kp], io_dt, kind="ExternalInput").ap()
    kv_ap = nc.dram_tensor("kv", [P, nk * (C + 1)], pv_dt, kind="ExternalInput").ap()
    out_ap = nc.dram_tensor("out", [P, nq * C], F32, kind="ExternalOutput").ap()
    with tile.TileContext(nc) as tc, ExitStack() as ctx:
        _emit(ctx, tc, out_ap, qt_ap, kt_ap, kv_ap, Qp, Kp)
    nc.compile()
    return nc


def shard_inputs(query, key_value, query_coors, key_value_coors):
    query = np.ascontiguousarray(np.asarray(query), dtype=np.float32)
    key_value = np.ascontiguousarray(np.asarray(key_value), dtype=np.float32)
    qc = np.asarray(query_coors).astype(np.int64)
    kc = np.asarray(key_value_coors).astype(np.int64)
    B = N_CORES
    ids = np.arange(B)
    qs = np.searchsorted(qc, ids, side="left")
    qe = np.searchsorted(qc, ids, side="right")
    ks = np.searchsorted(kc, ids, side="left")
    ke = np.searchsorted(kc, ids, side="right")
    qcnt, kcnt = qe - qs, ke - ks
    Qp = max(_round_up(int(qcnt.max()), P), P)
    Kp = max(_round_up(int(kcnt.max()), P), P)
    nk = Kp // P
    if S_MM == "bf16" or PV_MM == "bf16":
        import ml_dtypes
    in_maps = []
    for b in range(B):
        qsh = np.zeros((Qp, C), np.float32)
        qsh[: qcnt[b]] = query[qs[b]: qe[b]]
        kvsh = np.zeros((Kp, C + 1), np.float32)
        kvsh[: kcnt[b], :C] = key_value[ks[b]: ke[b]]
        kvsh[: kcnt[b], C] = 1.0
        qT = np.ascontiguousarray(qsh.T)
        kT = np.ascontiguousarray(kvsh[:, :C].T)
        kv_il = kvsh.reshape(nk, P, C + 1).transpose(1, 0, 2).reshape(P, nk * (C + 1))
        if S_MM == "bf16":
            qT = qT.astype(ml_dtypes.bfloat16)
            kT = kT.astype(ml_dtypes.bfloat16)
        if PV_MM == "bf16":
            kv_il = kv_il.astype(ml_dtypes.bfloat16)
        in_maps.append({
            "qT": np.ascontiguousarray(qT),
            "kT": np.ascontiguousarray(kT),
            "kv": np.ascontiguousarray(kv_il),
        })
    return in_maps, (qs, qe, qcnt), Qp, Kp


def kernel(query, key_value, query_coors, key_value_coors):
    in_maps, (qs, qe, qcnt), Qp, Kp = shard_inputs(
        query, key_value, query_coors, key_value_coors
    )
    nc = build_program(Qp, Kp)
    trace = bool(os.environ.get("XATTN_TRACE"))
    tcores = os.environ.get("XATTN_TRACE_CORES", "")
    if tcores:
        trace_cores = [int(x) for x in tcores.split(",")]
    else:
        trace_cores = list(range(N_CORES)) if trace else None
    res = run_bass_kernel_spmd(
        nc, in_maps, list(range(N_CORES)), trace=trace,
        trace_cores=trace_cores,
    )
    _LAST_RUN["exec_time_ns"] = res.exec_time_ns
    _LAST_RUN["mean_exec_time_ns"] = res.mean_exec_time_ns
    _LAST_RUN["trace"] = res.instructions_and_trace
    _LAST_RUN["results"] = res
    N1 = np.asarray(query).shape[0]
    nq = Qp // P
    out = np.zeros((N1, C), np.float32)
    for b in range(N_CORES):
        ob = res.results[b]["out"].reshape(P, nq, C).transpose(1, 0, 2).reshape(nq * P, C)
        out[qs[b]: qe[b]] = ob[: qcnt[b]]
    return out
